# revision 11
# baseline (speedup 1.0000x reference)
"""Bass/Trainium2 kernel for nn_BasicBlock_73933567033945 (CDConv / gnn_message_passing).

v2: TS=128 tiles (8/core). All partition shifts come from host-gathered
copies (pos/ori) or SBUF->SBUF partition-offset DMAs (h), eliminating the
shift matmuls. Geometry and the kern epilogue run as a handful of global
DVE ops over [128, (t,k,.)] views. The bilinear products run as DVE
tensor_mul over k-pairs (bf16), k-summed by PE transpose-accumulate into
PSUM, then Wk/W_out matmuls. All matmul operands are bf16 so FWL stays
enabled. Pure data parallel: no collectives.
"""
import numpy as np

B, L, C = 4, 2048, 128
N = B * L
W = 32
KC = 24
SEQ_L = 11
R = 12.0
WIN = 8
NEG_IN = 0.1
NEG_K = 0.2
NCORES = 8
NPC = N // NCORES          # 1024 nodes per core
TS = 128                   # output nodes per tile
NT = 8                     # tiles per core
NSLOT = 9                  # h halo slots of 128 (covers 1040 halo rows)
K17 = 2 * WIN + 1          # 17 window offsets
S_HALF = SEQ_L // 2

_PROG = {}


def _sidx(k):
    return int(np.clip(k - WIN, -S_HALF, S_HALF)) + S_HALF


def _build_program():
    import concourse.tile as tile
    from concourse import mybir, bacc
    from contextlib import ExitStack

    f32 = mybir.dt.float32
    bf16 = mybir.dt.bfloat16
    AF = mybir.ActivationFunctionType
    OP = mybir.AluOpType
    AX = mybir.AxisListType

    nc = bacc.Bacc("TRN2", target_bir_lowering=False, debug=False)

    def din(name, shape, dt=f32):
        return nc.dram_tensor(name, shape, dt, kind="ExternalInput").ap()

    xT_h = din("xT_h", [C, NSLOT * TS])               # x halo, channels on partitions
    po_sh = din("po_sh", [128, K17 * NT * 12])        # pre-shifted [pos|ori] halo
    xc = din("xc", [128, NT * C])                     # center x for identity add
    maskd = din("maskd", [128, NT * K17 * 8], bf16)   # edge mask expanded over delta8
    nclmp = din("nclmp", [128, NT])
    w_in = din("w_in", [C, W], bf16)
    ws_a = din("ws_a", [128, K17 * KC], bf16)
    ws_b = din("ws_b", [8, K17 * KC], bf16)
    wk_p = din("wk_p", [128, 6 * W], bf16)
    w_out = din("w_out", [W, C], bf16)
    ident = din("ident", [128, 128], bf16)
    w5r = din("w5r", [128, 3 * KC])
    b5r = din("b5r", [128, KC])
    alph1 = din("alph1", [128, 1])
    alph2 = din("alph2", [128, 1])
    y = nc.dram_tensor("y", [NPC, C], f32, kind="ExternalOutput").ap()

    with tile.TileContext(nc) as tc, ExitStack() as ctx:
        pers = ctx.enter_context(tc.tile_pool(name="pers", bufs=1))

        def load(ap_in, shape, tag, dt=f32):
            t = pers.tile(shape, dt, tag=tag)
            nc.sync.dma_start(t[:], ap_in)
            return t

        xT_sb = load(xT_h, [C, NSLOT * TS], "xT")
        po_sb = load(po_sh, [128, K17 * NT * 12], "po")
        xc_sb = load(xc, [128, NT * C], "xc")
        mask_sb = load(maskd, [128, NT * K17 * 8], "mask", bf16)
        ncl_sb = load(nclmp, [128, NT], "ncl")
        w_in_sb = load(w_in, [C, W], "w_in", bf16)
        ws_a_sb = load(ws_a, [128, K17 * KC], "ws_a", bf16)
        ws_b_sb = load(ws_b, [8, K17 * KC], "ws_b", bf16)
        wk_sb = load(wk_p, [128, 6 * W], "wk", bf16)
        w_out_sb = load(w_out, [W, C], "w_out", bf16)
        idb_sb = load(ident, [128, 128], "idb", bf16)
        w5r_sb = load(w5r, [128, 3 * KC], "w5r")
        b5r_sb = load(b5r, [128, KC], "b5r")
        a1_sb = load(alph1, [128, 1], "a1")
        a2_sb = load(alph2, [128, 1], "a2")

        # ------------- Phase A: h on all halo slots -----------------------
        xlr = pers.tile([C, NSLOT * TS], bf16, tag="xlr")
        nc.scalar.activation(xlr[:], xT_sb[:], AF.Prelu, bias=0.0, scale=1.0,
                             alpha=a1_sb[:, 0:1])
        h_slot = pers.tile([128, NSLOT * W], bf16, tag="h_slot")
        with tc.tile_pool(name="pAp", bufs=2, space="PSUM") as pAp:
            for j in range(NSLOT):
                hp = pAp.tile([TS, W], f32, tag="hp")
                nc.tensor.matmul(hp[:], xlr[:, TS * j:TS * (j + 1)], w_in_sb[:],
                                 start=True, stop=True)
                nc.scalar.activation(h_slot[:, W * j:W * (j + 1)], hp[:],
                                     AF.Prelu, bias=0.0, scale=1.0,
                                     alpha=a1_sb[:, 0:1])

        # ------------- h shifted copies (partition-offset DMAs) -----------
        # h_sh[p, k, t, w] = h[halo row 128t + p + k]
        h_sh = pers.tile([128, K17 * NT * W], bf16, tag="h_sh")
        for k in range(K17):
            blk = h_sh[:, k * NT * W:(k + 1) * NT * W]
            eng = nc.sync if k % 2 == 0 else nc.scalar
            eng.dma_start(blk[0:128 - k, :], h_slot[k:128, 0:NT * W])
            if k > 0:
                eng.dma_start(blk[128 - k:128, :], h_slot[0:k, W:(NT + 1) * W])

        # ------------- Global geometry ------------------------------------
        # po_sb views: [p, (k, t, 12)]
        pov = po_sb[:].rearrange("p (k t c) -> p k t c", t=NT, c=12)
        geo = ctx.enter_context(tc.tile_pool(name="geo", bufs=1))
        TK = NT * K17                         # 136 (t, k) pairs

        def tkv(tl, inner):
            return tl[:].rearrange("p (t k i) -> p t k i", k=K17, i=inner)

        pos_sh = pov[:, :, :, 0:3].rearrange("p k t c -> p t k c")
        pos_c = pov[:, 8, :, 0:3].unsqueeze(2).broadcast_to([128, NT, K17, 3])
        D = geo.tile([128, TK * 3], f32, tag="D")
        nc.vector.tensor_sub(tkv(D, 3), pos_sh, pos_c)
        sq = geo.tile([128, TK * 3], f32, tag="sq")
        nc.vector.tensor_mul(sq[:], D[:], D[:])
        d2 = geo.tile([128, TK], f32, tag="d2")
        nc.vector.tensor_reduce(d2[:], sq[:].rearrange("p (x a) -> p x a", a=3),
                                axis=AX.X, op=OP.add)
        da = geo.tile([128, TK * 8], bf16, tag="da")
        dav = da[:].rearrange("p (x i) -> p x i", i=8)
        # dist/R -> slot 6 ; raw dist for direction
        nc.scalar.activation(dav[:, :, 6], d2[:], AF.Sqrt, bias=0.0,
                             scale=1.0 / (R * R))
        dist = geo.tile([128, TK], f32, tag="dist")
        nc.scalar.activation(dist[:], d2[:], AF.Sqrt, bias=0.0, scale=1.0)
        nc.vector.tensor_scalar_add(dist[:], dist[:], 1e-9)
        rec = geo.tile([128, TK], f32, tag="rec")
        nc.vector.reciprocal(rec[:], dist[:])
        dirn = geo.tile([128, TK * 3], f32, tag="dirn")
        nc.vector.tensor_mul(
            dirn[:].rearrange("p (x a) -> p x a", a=3),
            D[:].rearrange("p (x a) -> p x a", a=3),
            rec[:].unsqueeze(-1).broadcast_to([128, TK, 3]))
        # local_a = sum_b Ri[a,b] * dirn[b]
        ori_c = pov[:, 8, :, 3:12].rearrange("p t (a b) -> p t a b", b=3)
        lm = geo.tile([128, TK * 9], f32, tag="lm")
        lmv = lm[:].rearrange("p (t k a b) -> p t k a b", k=K17, a=3, b=3)
        nc.vector.tensor_mul(
            lmv,
            ori_c.unsqueeze(2).broadcast_to([128, NT, K17, 3, 3]),
            tkv(dirn, 3).unsqueeze(3).broadcast_to([128, NT, K17, 3, 3]))
        with nc.allow_low_precision(reason="3-term sums into bf16 delta"):
            nc.vector.tensor_reduce(
                dav[:, :, 0:3],
                lm[:].rearrange("p (x a b) -> p x a b", a=3, b=3),
                axis=AX.X, op=OP.add)
        # ofeat_a = sum_b Ri[a,b] * Rj[a,b]
        ori_sh = pov[:, :, :, 3:12].rearrange("p k t c -> p t k c") \
            .rearrange("p t k (a b) -> p t k a b", b=3)
        ofm = geo.tile([128, TK * 9], f32, tag="ofm")
        ofmv = ofm[:].rearrange("p (t k a b) -> p t k a b", k=K17, a=3, b=3)
        nc.vector.tensor_mul(
            ofmv, ori_sh,
            ori_c.unsqueeze(2).broadcast_to([128, NT, K17, 3, 3]))
        with nc.allow_low_precision(reason="3-term sums into bf16 delta"):
            nc.vector.tensor_reduce(
                dav[:, :, 3:6],
                ofm[:].rearrange("p (x a b) -> p x a b", a=3, b=3),
                axis=AX.X, op=OP.add)
        nc.vector.memset(dav[:, :, 7], 1.0)
        # chain-boundary mask (pre-expanded over the 8 delta slots)
        nc.vector.tensor_mul(da[:], da[:], mask_sb[:])

        # ------------- kern = lrelu(da @ WS) per tile ---------------------
        kern = pers.tile([128, NT * K17 * KC], bf16, tag="kern")
        with tc.tile_pool(name="kw", bufs=2) as kw, \
             tc.tile_pool(name="kp", bufs=2, space="PSUM") as kp:
            for t in range(NT):
                dsl = da[:, t * K17 * 8:(t + 1) * K17 * 8]
                dT_p = kp.tile([128, 256], bf16, tag="dT")
                nc.tensor.matmul(dT_p[:, 0:128], dsl[:, 0:128], idb_sb[:],
                                 is_transpose=True, start=True, stop=False,
                                 skip_group_check=True)
                nc.tensor.matmul(dT_p[0:8, 128:256], dsl[:, 128:136], idb_sb[:],
                                 is_transpose=True, start=False, stop=True,
                                 skip_group_check=True)
                dT = kw.tile([128, 256], bf16, tag="dT_sb")
                nc.scalar.copy(dT[:], dT_p[:])
                pre_p = kp.tile([128, K17 * KC], f32, tag="pre")
                nc.tensor.matmul(pre_p[:], dT[:, 0:128], ws_a_sb[:], start=True,
                                 stop=False, skip_group_check=True)
                nc.tensor.matmul(pre_p[:], dT[0:8, 128:256], ws_b_sb[:],
                                 start=False, stop=True, skip_group_check=True)
                nc.scalar.activation(kern[:, t * K17 * KC:(t + 1) * K17 * KC],
                                     pre_p[:], AF.Prelu, bias=0.0, scale=1.0,
                                     alpha=a2_sb[:, 0:1])

        # ------------- self-edge compensation (global) --------------------
        rn = geo.tile([128, NT * 3], f32, tag="rn")
        nc.vector.tensor_reduce(
            rn[:].rearrange("p (t a) -> p t a", a=3),
            ofmv[:, :, 8], axis=AX.X, op=OP.add)
        ps0 = geo.tile([128, NT * KC * 3], f32, tag="ps0")
        nc.vector.tensor_mul(
            ps0[:].rearrange("p (t c a) -> p t c a", c=KC, a=3),
            w5r_sb[:].rearrange("p (a c) -> p c a", a=3).unsqueeze(1)
                .broadcast_to([128, NT, KC, 3]),
            rn[:].rearrange("p (t a) -> p t a", a=3).unsqueeze(2)
                .broadcast_to([128, NT, KC, 3]))
        pself = geo.tile([128, NT * KC], f32, tag="pself")
        nc.vector.tensor_reduce(pself[:].rearrange("p (t c) -> p t c", c=KC),
                                ps0[:].rearrange("p (t c a) -> p t c a", a=3, c=KC),
                                axis=AX.X, op=OP.add)
        nc.vector.tensor_add(
            pself[:].rearrange("p (t c) -> p t c", c=KC),
            pself[:].rearrange("p (t c) -> p t c", c=KC),
            b5r_sb[:].unsqueeze(1).broadcast_to([128, NT, KC]))
        kself = geo.tile([128, NT * KC], f32, tag="kself")
        nc.vector.scalar_tensor_tensor(kself[:], pself[:], NEG_K, pself[:],
                                       OP.mult, OP.max)
        nc.vector.tensor_mul(
            kself[:].rearrange("p (t c) -> p t c", c=KC),
            kself[:].rearrange("p (t c) -> p t c", c=KC),
            ncl_sb[:].unsqueeze(-1).broadcast_to([128, NT, KC]))
        # kern k=8 slice += kself
        k8 = kern[:].rearrange("p (t k c) -> p t k c", k=K17, c=KC)[:, :, 8, :]
        nc.vector.tensor_add(k8, k8, kself[:].rearrange("p (t c) -> p t c", c=KC))

        # ------------- Core loop: products -> transpose-accum -> Wk -------
        wrk = ctx.enter_context(tc.tile_pool(name="wrk", bufs=3))
        tpool = ctx.enter_context(tc.tile_pool(name="tmp", bufs=10))
        psA = ctx.enter_context(tc.tile_pool(name="psA", bufs=2, space="PSUM"))
        psB = ctx.enter_context(tc.tile_pool(name="psB", bufs=2, space="PSUM"))

        kern_v = kern[:].rearrange("p (t k c) -> p t k c", k=K17, c=KC)
        hsh_v = h_sh[:].rearrange("p (k t w) -> p k t w", t=NT, w=W)
        # products: k0-11 direct DVE pairs (1x), k12-16 single-k on GpSimd.
        EXP_PAIRS = []
        DIR_PAIRS = [(0, 2), (2, 4), (4, 6), (6, 8), (8, 10), (10, 12)]
        GP_SINGLE = [12, 13, 14, 15, 16]
        for t in range(NT):
            aggT_p = psA.tile([128, 768], f32, tag="aggT")
            tms = {}

            def hview(ka, kb):
                return hsh_v[:, ka:kb, t, :].unsqueeze(2) \
                    .broadcast_to([128, kb - ka, KC, W])

            def kview(ka, kb):
                return kern_v[:, t, ka:kb, :].unsqueeze(-1) \
                    .broadcast_to([128, kb - ka, KC, W])

            for (ka, kb) in EXP_PAIRS:
                nk = kb - ka
                krep = tpool.tile([128, nk * KC * W], bf16, tag="krep")
                krv = krep[:].rearrange("p (k c w) -> p k c w", c=KC, w=W)
                nc.scalar.copy(krv, kview(ka, kb))
                tm = tpool.tile([128, nk * KC * W], bf16, tag="tm")
                nc.vector.tensor_mul(
                    tm[:].rearrange("p (k c w) -> p k c w", c=KC, w=W),
                    hview(ka, kb), krv)
                tms[ka] = tm
            for (ka, kb) in DIR_PAIRS:
                nk = kb - ka
                tm = tpool.tile([128, nk * KC * W], bf16, tag="tm")
                nc.vector.tensor_mul(
                    tm[:].rearrange("p (k c w) -> p k c w", c=KC, w=W),
                    hview(ka, kb), kview(ka, kb))
                tms[ka] = tm
            for k in GP_SINGLE:
                tm = tpool.tile([128, KC * W], bf16, tag="tmg")
                nc.gpsimd.tensor_mul(
                    tm[:].rearrange("p (c w) -> p c w", w=W),
                    hsh_v[:, k, t, :].unsqueeze(1).broadcast_to([128, KC, W]),
                    kern_v[:, t, k, :].unsqueeze(-1).broadcast_to([128, KC, W]))
                tms[k] = tm

            # PE transpose-accumulate in k order
            cover = EXP_PAIRS + DIR_PAIRS + [(k, k + 1) for k in GP_SINGLE]
            for (ka, kb) in cover:
                tm = tms[ka]
                for kk in range(kb - ka):
                    k = ka + kk
                    for b in range(6):
                        nc.tensor.matmul(
                            aggT_p[:, 128 * b:128 * (b + 1)],
                            tm[:, 128 * (6 * kk + b):128 * (6 * kk + b + 1)],
                            idb_sb[:],
                            start=(k == 0 and b in (0, 4)),
                            stop=(k == 16 and b in (3, 5)),
                            skip_group_check=True)
            aggT = wrk.tile([128, 768], bf16, tag="aggT_sb")
            nc.scalar.copy(aggT[:], aggT_p[:])

            co_p = psB.tile([128, 256], f32, tag="co")
            for b in range(6):
                nc.tensor.matmul(co_p[0:W, 0:128], wk_sb[:, W * b:W * (b + 1)],
                                 aggT[:, 128 * b:128 * (b + 1)],
                                 start=(b == 0), stop=(b == 5),
                                 skip_group_check=True)
            convL = wrk.tile([W, 128], bf16, tag="convL")
            nc.scalar.activation(convL[:], co_p[0:W, 0:128], AF.Prelu, bias=0.0,
                                 scale=1.0, alpha=a1_sb[0:W, 0:1])
            nc.tensor.matmul(co_p[:, 128:256], convL[:], w_out_sb[:],
                             start=True, stop=True, skip_group_check=True)
            out_sb = wrk.tile([128, C], f32, tag="out_sb")
            nc.vector.tensor_add(out_sb[:], co_p[:, 128:256],
                                 xc_sb[:, C * t:C * (t + 1)])
            nc.sync.dma_start(y[TS * t:TS * (t + 1), :], out_sb[:])

    nc.compile()
    return nc


def _expected_src_dst():
    i = np.arange(N)
    offs = np.arange(-WIN, WIN + 1)
    j = i[:, None] + offs[None, :]
    valid = ((j // L) == (i[:, None] // L)) & (j >= 0) & (j < N)
    j = np.where(valid, j, i[:, None])
    dst = np.repeat(i, offs.size).astype(np.int32)
    src = j.reshape(-1).astype(np.int32)
    return src, dst


def _host_inputs(x, pos, ori, W_in, Ws0, bs0, Wk, W_out):
    from ml_dtypes import bfloat16
    xf = np.ascontiguousarray(x.reshape(N, C), np.float32)
    pos = np.asarray(pos, np.float32)
    ori = np.asarray(ori, np.float32)

    WS = np.zeros((136, K17 * KC), np.float32)
    for k in range(K17):
        s = _sidx(k)
        WS[8 * k:8 * k + 7, KC * k:KC * (k + 1)] = Ws0[s]
        WS[8 * k + 7, KC * k:KC * (k + 1)] = bs0[s]
    wk_p = np.zeros((128, 6 * W), np.float32)
    for b in range(6):
        wk_p[:, W * b:W * (b + 1)] = Wk[128 * b:128 * (b + 1), :]
    w5r = np.tile(Ws0[5][3:6].reshape(1, 3 * KC), (128, 1)).astype(np.float32)
    b5r = np.tile(bs0[5].reshape(1, KC), (128, 1)).astype(np.float32)

    def b16(a):
        return np.ascontiguousarray(a).astype(bfloat16)

    common = dict(
        w_in=b16(W_in),
        ws_a=b16(WS[0:128]),
        ws_b=b16(WS[128:136]),
        wk_p=b16(wk_p),
        w_out=b16(W_out),
        ident=b16(np.eye(128, dtype=np.float32)),
        w5r=w5r, b5r=b5r,
        alph1=np.full((128, 1), NEG_IN, np.float32),
        alph2=np.full((128, 1), NEG_K, np.float32),
    )

    po_full = np.concatenate([pos, ori], axis=1)  # [N, 12]
    in_maps = []
    for ci in range(NCORES):
        s0 = ci * NPC
        # x halo transposed: slot j col p -> node s0 - 8 + 128j + p
        g = s0 - WIN + np.arange(NSLOT * TS)
        ok = (g >= 0) & (g < N)
        gi = np.clip(g, 0, N - 1)
        x_halo = np.where(ok[:, None], xf[gi], 0.0).astype(np.float32)
        xT_h = np.ascontiguousarray(x_halo.T)  # [C, NSLOT*TS]

        # pre-shifted pos/ori: po_sh[p, k, t, :] = po[s0 - 8 + 128t + p + k]
        p_ = np.arange(128)
        k_ = np.arange(K17)
        t_ = np.arange(NT)
        idx = (s0 - WIN + 128 * t_[None, None, :] + p_[:, None, None]
               + k_[None, :, None])                       # [128, K17, NT]
        okp = (idx >= 0) & (idx < N)
        po_g = np.where(okp[..., None], po_full[np.clip(idx, 0, N - 1)], 0.0)
        po_sh = po_g.reshape(128, K17 * NT * 12).astype(np.float32)

        xc = xf[s0:s0 + NPC].reshape(NT, 128, C).transpose(1, 0, 2) \
            .reshape(128, NT * C).astype(np.float32)

        # chain-boundary masks
        n_ = s0 + 128 * t_[None, :] + p_[:, None]          # [128, NT]
        off = n_ % L
        kk = np.arange(-WIN, WIN + 1)
        v = ((off[..., None] + kk) >= 0) & ((off[..., None] + kk) < L)  # [128,NT,K17]
        mask = np.broadcast_to(v[..., None].astype(np.float32),
                               (128, NT, K17, 8)).reshape(128, NT * K17 * 8)
        ncl = (K17 - v.sum(-1)).astype(np.float32)

        in_maps.append(dict(
            xT_h=xT_h, po_sh=po_sh, xc=xc,
            maskd=mask.astype(bfloat16), nclmp=ncl, **common))
    return in_maps


def kernel(x, pos, seq, ori, W_in, Ws0, bs0, Wk, W_out, src, dst):
    exp_src, exp_dst = _expected_src_dst()
    assert np.array_equal(np.asarray(src), exp_src), "unexpected src graph"
    assert np.array_equal(np.asarray(dst), exp_dst), "unexpected dst graph"

    from concourse.bass_utils import run_bass_kernel_spmd

    if "nc" not in _PROG:
        _PROG["nc"] = _build_program()
    nc = _PROG["nc"]

    in_maps = _host_inputs(np.asarray(x), np.asarray(pos), np.asarray(ori),
                           np.asarray(W_in), np.asarray(Ws0), np.asarray(bs0),
                           np.asarray(Wk), np.asarray(W_out))
    res = run_bass_kernel_spmd(nc, in_maps, list(range(NCORES)))
    out = np.concatenate([res.results[i]["y"] for i in range(NCORES)], axis=0)
    return out.reshape(B, L, C).astype(np.float32)


# revision 12
# speedup vs baseline: 1.0333x; 1.0333x over previous
"""Bass/Trainium2 kernel for nn_BasicBlock_73933567033945 (CDConv / gnn_message_passing).

v2: TS=128 tiles (8/core). All partition shifts come from host-gathered
copies (pos/ori) or SBUF->SBUF partition-offset DMAs (h), eliminating the
shift matmuls. Geometry and the kern epilogue run as a handful of global
DVE ops over [128, (t,k,.)] views. The bilinear products run as DVE
tensor_mul over k-pairs (bf16), k-summed by PE transpose-accumulate into
PSUM, then Wk/W_out matmuls. All matmul operands are bf16 so FWL stays
enabled. Pure data parallel: no collectives.
"""
import numpy as np

B, L, C = 4, 2048, 128
N = B * L
W = 32
KC = 24
SEQ_L = 11
R = 12.0
WIN = 8
NEG_IN = 0.1
NEG_K = 0.2
NCORES = 8
NPC = N // NCORES          # 1024 nodes per core
TS = 128                   # output nodes per tile
NT = 8                     # tiles per core
NSLOT = 9                  # h halo slots of 128 (covers 1040 halo rows)
K17 = 2 * WIN + 1          # 17 window offsets
S_HALF = SEQ_L // 2

_PROG = {}


def _sidx(k):
    return int(np.clip(k - WIN, -S_HALF, S_HALF)) + S_HALF


def _build_program():
    import concourse.tile as tile
    from concourse import mybir, bacc
    from contextlib import ExitStack

    f32 = mybir.dt.float32
    bf16 = mybir.dt.bfloat16
    AF = mybir.ActivationFunctionType
    OP = mybir.AluOpType
    AX = mybir.AxisListType

    nc = bacc.Bacc("TRN2", target_bir_lowering=False, debug=False)

    def din(name, shape, dt=f32):
        return nc.dram_tensor(name, shape, dt, kind="ExternalInput").ap()

    xT_h = din("xT_h", [C, NSLOT * TS])               # x halo, channels on partitions
    po_sh = din("po_sh", [128, K17 * NT * 12])        # pre-shifted [pos|ori] halo
    xc = din("xc", [128, NT * C])                     # center x for identity add
    maskd = din("maskd", [128, NT * K17 * 8], bf16)   # edge mask expanded over delta8
    nclmp = din("nclmp", [128, NT])
    w_in = din("w_in", [C, W], bf16)
    ws_a = din("ws_a", [128, K17 * KC], bf16)
    ws_b = din("ws_b", [8, K17 * KC], bf16)
    wk_p = din("wk_p", [128, 6 * W], bf16)
    w_out = din("w_out", [W, C], bf16)
    ident = din("ident", [128, 128], bf16)
    w5r = din("w5r", [128, 3 * KC])
    b5r = din("b5r", [128, KC])
    alph1 = din("alph1", [128, 1])
    alph2 = din("alph2", [128, 1])
    y = nc.dram_tensor("y", [NPC, C], f32, kind="ExternalOutput").ap()

    with tile.TileContext(nc) as tc, ExitStack() as ctx:
        pers = ctx.enter_context(tc.tile_pool(name="pers", bufs=1))

        def load(ap_in, shape, tag, dt=f32):
            t = pers.tile(shape, dt, tag=tag)
            nc.sync.dma_start(t[:], ap_in)
            return t

        xT_sb = load(xT_h, [C, NSLOT * TS], "xT")
        po_sb = load(po_sh, [128, K17 * NT * 12], "po")
        xc_sb = load(xc, [128, NT * C], "xc")
        mask_sb = load(maskd, [128, NT * K17 * 8], "mask", bf16)
        ncl_sb = load(nclmp, [128, NT], "ncl")
        w_in_sb = load(w_in, [C, W], "w_in", bf16)
        ws_a_sb = load(ws_a, [128, K17 * KC], "ws_a", bf16)
        ws_b_sb = load(ws_b, [8, K17 * KC], "ws_b", bf16)
        wk_sb = load(wk_p, [128, 6 * W], "wk", bf16)
        w_out_sb = load(w_out, [W, C], "w_out", bf16)
        idb_sb = load(ident, [128, 128], "idb", bf16)
        w5r_sb = load(w5r, [128, 3 * KC], "w5r")
        b5r_sb = load(b5r, [128, KC], "b5r")
        a1_sb = load(alph1, [128, 1], "a1")
        a2_sb = load(alph2, [128, 1], "a2")

        # ------------- Phase A: h on all halo slots -----------------------
        xlr = pers.tile([C, NSLOT * TS], bf16, tag="xlr")
        nc.scalar.activation(xlr[:], xT_sb[:], AF.Prelu, bias=0.0, scale=1.0,
                             alpha=a1_sb[:, 0:1])
        h_slot = pers.tile([128, NSLOT * W], bf16, tag="h_slot")
        with tc.tile_pool(name="pAp", bufs=2, space="PSUM") as pAp:
            for j in range(NSLOT):
                hp = pAp.tile([TS, W], f32, tag="hp")
                nc.tensor.matmul(hp[:], xlr[:, TS * j:TS * (j + 1)], w_in_sb[:],
                                 start=True, stop=True)
                nc.scalar.activation(h_slot[:, W * j:W * (j + 1)], hp[:],
                                     AF.Prelu, bias=0.0, scale=1.0,
                                     alpha=a1_sb[:, 0:1])

        # ------------- h shifted copies (partition-offset DMAs) -----------
        # h_sh[p, k, t, w] = h[halo row 128t + p + k]
        h_sh = pers.tile([128, K17 * NT * W], bf16, tag="h_sh")
        for k in range(K17):
            blk = h_sh[:, k * NT * W:(k + 1) * NT * W]
            eng = nc.sync if k % 2 == 0 else nc.scalar
            eng.dma_start(blk[0:128 - k, :], h_slot[k:128, 0:NT * W])
            if k > 0:
                eng.dma_start(blk[128 - k:128, :], h_slot[0:k, W:(NT + 1) * W])

        # ------------- Global geometry ------------------------------------
        # po_sb views: [p, (k, t, 12)]
        pov = po_sb[:].rearrange("p (k t c) -> p k t c", t=NT, c=12)
        geo = ctx.enter_context(tc.tile_pool(name="geo", bufs=1))
        TK = NT * K17                         # 136 (t, k) pairs

        def tkv(tl, inner):
            return tl[:].rearrange("p (t k i) -> p t k i", k=K17, i=inner)

        pos_sh = pov[:, :, :, 0:3].rearrange("p k t c -> p t k c")
        pos_c = pov[:, 8, :, 0:3].unsqueeze(2).broadcast_to([128, NT, K17, 3])
        D = geo.tile([128, TK * 3], f32, tag="D")
        nc.vector.tensor_sub(tkv(D, 3), pos_sh, pos_c)
        sq = geo.tile([128, TK * 3], f32, tag="sq")
        nc.vector.tensor_mul(sq[:], D[:], D[:])
        d2 = geo.tile([128, TK], f32, tag="d2")
        nc.vector.tensor_reduce(d2[:], sq[:].rearrange("p (x a) -> p x a", a=3),
                                axis=AX.X, op=OP.add)
        da = geo.tile([128, TK * 8], bf16, tag="da")
        dav = da[:].rearrange("p (x i) -> p x i", i=8)
        # dist/R -> slot 6 ; raw dist for direction
        nc.scalar.activation(dav[:, :, 6], d2[:], AF.Sqrt, bias=0.0,
                             scale=1.0 / (R * R))
        dist = geo.tile([128, TK], f32, tag="dist")
        nc.scalar.activation(dist[:], d2[:], AF.Sqrt, bias=0.0, scale=1.0)
        nc.vector.tensor_scalar_add(dist[:], dist[:], 1e-9)
        rec = geo.tile([128, TK], f32, tag="rec")
        nc.vector.reciprocal(rec[:], dist[:])
        dirn = geo.tile([128, TK * 3], f32, tag="dirn")
        nc.vector.tensor_mul(
            dirn[:].rearrange("p (x a) -> p x a", a=3),
            D[:].rearrange("p (x a) -> p x a", a=3),
            rec[:].unsqueeze(-1).broadcast_to([128, TK, 3]))
        # local_a = sum_b Ri[a,b] * dirn[b]
        ori_c = pov[:, 8, :, 3:12].rearrange("p t (a b) -> p t a b", b=3)
        lm = geo.tile([128, TK * 9], f32, tag="lm")
        lmv = lm[:].rearrange("p (t k a b) -> p t k a b", k=K17, a=3, b=3)
        nc.vector.tensor_mul(
            lmv,
            ori_c.unsqueeze(2).broadcast_to([128, NT, K17, 3, 3]),
            tkv(dirn, 3).unsqueeze(3).broadcast_to([128, NT, K17, 3, 3]))
        with nc.allow_low_precision(reason="3-term sums into bf16 delta"):
            nc.vector.tensor_reduce(
                dav[:, :, 0:3],
                lm[:].rearrange("p (x a b) -> p x a b", a=3, b=3),
                axis=AX.X, op=OP.add)
        # ofeat_a = sum_b Ri[a,b] * Rj[a,b]
        ori_sh = pov[:, :, :, 3:12].rearrange("p k t c -> p t k c") \
            .rearrange("p t k (a b) -> p t k a b", b=3)
        ofm = geo.tile([128, TK * 9], f32, tag="ofm")
        ofmv = ofm[:].rearrange("p (t k a b) -> p t k a b", k=K17, a=3, b=3)
        nc.vector.tensor_mul(
            ofmv, ori_sh,
            ori_c.unsqueeze(2).broadcast_to([128, NT, K17, 3, 3]))
        with nc.allow_low_precision(reason="3-term sums into bf16 delta"):
            nc.vector.tensor_reduce(
                dav[:, :, 3:6],
                ofm[:].rearrange("p (x a b) -> p x a b", a=3, b=3),
                axis=AX.X, op=OP.add)
        nc.vector.memset(dav[:, :, 7], 1.0)
        # chain-boundary mask (pre-expanded over the 8 delta slots)
        nc.vector.tensor_mul(da[:], da[:], mask_sb[:])

        # ------------- kern = lrelu(da @ WS) per tile ---------------------
        kern = pers.tile([128, NT * K17 * KC], bf16, tag="kern")
        with tc.tile_pool(name="kw", bufs=2) as kw, \
             tc.tile_pool(name="kp", bufs=2, space="PSUM") as kp:
            for t in range(NT):
                dsl = da[:, t * K17 * 8:(t + 1) * K17 * 8]
                dT_p = kp.tile([128, 256], bf16, tag="dT")
                nc.tensor.matmul(dT_p[:, 0:128], dsl[:, 0:128], idb_sb[:],
                                 is_transpose=True, start=True, stop=False,
                                 skip_group_check=True)
                nc.tensor.matmul(dT_p[0:8, 128:256], dsl[:, 128:136], idb_sb[:],
                                 is_transpose=True, start=False, stop=True,
                                 skip_group_check=True)
                dT = kw.tile([128, 256], bf16, tag="dT_sb")
                nc.scalar.copy(dT[:], dT_p[:])
                pre_p = kp.tile([128, K17 * KC], f32, tag="pre")
                nc.tensor.matmul(pre_p[:], dT[:, 0:128], ws_a_sb[:], start=True,
                                 stop=False, skip_group_check=True)
                nc.tensor.matmul(pre_p[:], dT[0:8, 128:256], ws_b_sb[:],
                                 start=False, stop=True, skip_group_check=True)
                nc.scalar.activation(kern[:, t * K17 * KC:(t + 1) * K17 * KC],
                                     pre_p[:], AF.Prelu, bias=0.0, scale=1.0,
                                     alpha=a2_sb[:, 0:1])

        # ------------- self-edge compensation (global) --------------------
        rn = geo.tile([128, NT * 3], f32, tag="rn")
        nc.vector.tensor_reduce(
            rn[:].rearrange("p (t a) -> p t a", a=3),
            ofmv[:, :, 8], axis=AX.X, op=OP.add)
        ps0 = geo.tile([128, NT * KC * 3], f32, tag="ps0")
        nc.vector.tensor_mul(
            ps0[:].rearrange("p (t c a) -> p t c a", c=KC, a=3),
            w5r_sb[:].rearrange("p (a c) -> p c a", a=3).unsqueeze(1)
                .broadcast_to([128, NT, KC, 3]),
            rn[:].rearrange("p (t a) -> p t a", a=3).unsqueeze(2)
                .broadcast_to([128, NT, KC, 3]))
        pself = geo.tile([128, NT * KC], f32, tag="pself")
        nc.vector.tensor_reduce(pself[:].rearrange("p (t c) -> p t c", c=KC),
                                ps0[:].rearrange("p (t c a) -> p t c a", a=3, c=KC),
                                axis=AX.X, op=OP.add)
        nc.vector.tensor_add(
            pself[:].rearrange("p (t c) -> p t c", c=KC),
            pself[:].rearrange("p (t c) -> p t c", c=KC),
            b5r_sb[:].unsqueeze(1).broadcast_to([128, NT, KC]))
        kself = geo.tile([128, NT * KC], f32, tag="kself")
        nc.vector.scalar_tensor_tensor(kself[:], pself[:], NEG_K, pself[:],
                                       OP.mult, OP.max)
        nc.vector.tensor_mul(
            kself[:].rearrange("p (t c) -> p t c", c=KC),
            kself[:].rearrange("p (t c) -> p t c", c=KC),
            ncl_sb[:].unsqueeze(-1).broadcast_to([128, NT, KC]))
        # kern k=8 slice += kself
        k8 = kern[:].rearrange("p (t k c) -> p t k c", k=K17, c=KC)[:, :, 8, :]
        nc.vector.tensor_add(k8, k8, kself[:].rearrange("p (t c) -> p t c", c=KC))

        # ------------- Core loop: products -> transpose-accum -> Wk -------
        wrk = ctx.enter_context(tc.tile_pool(name="wrk", bufs=3))
        tpool = ctx.enter_context(tc.tile_pool(name="tmp", bufs=10))
        psA = ctx.enter_context(tc.tile_pool(name="psA", bufs=2, space="PSUM"))
        psB = ctx.enter_context(tc.tile_pool(name="psB", bufs=2, space="PSUM"))

        kern_v = kern[:].rearrange("p (t k c) -> p t k c", k=K17, c=KC)
        hsh_v = h_sh[:].rearrange("p (k t w) -> p k t w", t=NT, w=W)
        # products: k0-11 direct DVE pairs (1x), k12-16 single-k on GpSimd.
        EXP_PAIRS = []
        DIR_PAIRS = [(0, 2), (2, 4), (4, 6), (6, 8), (8, 10), (10, 12), (12, 13)]
        GP_SINGLE = [13, 14, 15, 16]
        for t in range(NT):
            aggT_p = psA.tile([128, 768], f32, tag="aggT")
            tms = {}

            def hview(ka, kb):
                return hsh_v[:, ka:kb, t, :].unsqueeze(2) \
                    .broadcast_to([128, kb - ka, KC, W])

            def kview(ka, kb):
                return kern_v[:, t, ka:kb, :].unsqueeze(-1) \
                    .broadcast_to([128, kb - ka, KC, W])

            for (ka, kb) in EXP_PAIRS:
                nk = kb - ka
                krep = tpool.tile([128, nk * KC * W], bf16, tag="krep")
                krv = krep[:].rearrange("p (k c w) -> p k c w", c=KC, w=W)
                nc.scalar.copy(krv, kview(ka, kb))
                tm = tpool.tile([128, nk * KC * W], bf16, tag="tm")
                nc.vector.tensor_mul(
                    tm[:].rearrange("p (k c w) -> p k c w", c=KC, w=W),
                    hview(ka, kb), krv)
                tms[ka] = tm
            for (ka, kb) in DIR_PAIRS:
                nk = kb - ka
                tm = tpool.tile([128, nk * KC * W], bf16, tag="tm")
                nc.vector.tensor_mul(
                    tm[:].rearrange("p (k c w) -> p k c w", c=KC, w=W),
                    hview(ka, kb), kview(ka, kb))
                tms[ka] = tm
            for k in GP_SINGLE:
                tm = tpool.tile([128, KC * W], bf16, tag="tmg")
                nc.gpsimd.tensor_mul(
                    tm[:].rearrange("p (c w) -> p c w", w=W),
                    hsh_v[:, k, t, :].unsqueeze(1).broadcast_to([128, KC, W]),
                    kern_v[:, t, k, :].unsqueeze(-1).broadcast_to([128, KC, W]))
                tms[k] = tm

            # PE transpose-accumulate in k order
            cover = EXP_PAIRS + DIR_PAIRS + [(k, k + 1) for k in GP_SINGLE]
            for (ka, kb) in cover:
                tm = tms[ka]
                for kk in range(kb - ka):
                    k = ka + kk
                    for b in range(6):
                        nc.tensor.matmul(
                            aggT_p[:, 128 * b:128 * (b + 1)],
                            tm[:, 128 * (6 * kk + b):128 * (6 * kk + b + 1)],
                            idb_sb[:],
                            start=(k == 0 and b in (0, 4)),
                            stop=(k == 16 and b in (3, 5)),
                            skip_group_check=True)
            aggT = wrk.tile([128, 768], bf16, tag="aggT_sb")
            nc.scalar.copy(aggT[:], aggT_p[:])

            co_p = psB.tile([128, 256], f32, tag="co")
            for b in range(6):
                nc.tensor.matmul(co_p[0:W, 0:128], wk_sb[:, W * b:W * (b + 1)],
                                 aggT[:, 128 * b:128 * (b + 1)],
                                 start=(b == 0), stop=(b == 5),
                                 skip_group_check=True)
            convL = wrk.tile([W, 128], bf16, tag="convL")
            nc.scalar.activation(convL[:], co_p[0:W, 0:128], AF.Prelu, bias=0.0,
                                 scale=1.0, alpha=a1_sb[0:W, 0:1])
            nc.tensor.matmul(co_p[:, 128:256], convL[:], w_out_sb[:],
                             start=True, stop=True, skip_group_check=True)
            out_sb = wrk.tile([128, C], f32, tag="out_sb")
            nc.vector.tensor_add(out_sb[:], co_p[:, 128:256],
                                 xc_sb[:, C * t:C * (t + 1)])
            nc.sync.dma_start(y[TS * t:TS * (t + 1), :], out_sb[:])

    nc.compile()
    return nc


def _expected_src_dst():
    i = np.arange(N)
    offs = np.arange(-WIN, WIN + 1)
    j = i[:, None] + offs[None, :]
    valid = ((j // L) == (i[:, None] // L)) & (j >= 0) & (j < N)
    j = np.where(valid, j, i[:, None])
    dst = np.repeat(i, offs.size).astype(np.int32)
    src = j.reshape(-1).astype(np.int32)
    return src, dst


def _host_inputs(x, pos, ori, W_in, Ws0, bs0, Wk, W_out):
    from ml_dtypes import bfloat16
    xf = np.ascontiguousarray(x.reshape(N, C), np.float32)
    pos = np.asarray(pos, np.float32)
    ori = np.asarray(ori, np.float32)

    WS = np.zeros((136, K17 * KC), np.float32)
    for k in range(K17):
        s = _sidx(k)
        WS[8 * k:8 * k + 7, KC * k:KC * (k + 1)] = Ws0[s]
        WS[8 * k + 7, KC * k:KC * (k + 1)] = bs0[s]
    wk_p = np.zeros((128, 6 * W), np.float32)
    for b in range(6):
        wk_p[:, W * b:W * (b + 1)] = Wk[128 * b:128 * (b + 1), :]
    w5r = np.tile(Ws0[5][3:6].reshape(1, 3 * KC), (128, 1)).astype(np.float32)
    b5r = np.tile(bs0[5].reshape(1, KC), (128, 1)).astype(np.float32)

    def b16(a):
        return np.ascontiguousarray(a).astype(bfloat16)

    common = dict(
        w_in=b16(W_in),
        ws_a=b16(WS[0:128]),
        ws_b=b16(WS[128:136]),
        wk_p=b16(wk_p),
        w_out=b16(W_out),
        ident=b16(np.eye(128, dtype=np.float32)),
        w5r=w5r, b5r=b5r,
        alph1=np.full((128, 1), NEG_IN, np.float32),
        alph2=np.full((128, 1), NEG_K, np.float32),
    )

    po_full = np.concatenate([pos, ori], axis=1)  # [N, 12]
    in_maps = []
    for ci in range(NCORES):
        s0 = ci * NPC
        # x halo transposed: slot j col p -> node s0 - 8 + 128j + p
        g = s0 - WIN + np.arange(NSLOT * TS)
        ok = (g >= 0) & (g < N)
        gi = np.clip(g, 0, N - 1)
        x_halo = np.where(ok[:, None], xf[gi], 0.0).astype(np.float32)
        xT_h = np.ascontiguousarray(x_halo.T)  # [C, NSLOT*TS]

        # pre-shifted pos/ori: po_sh[p, k, t, :] = po[s0 - 8 + 128t + p + k]
        p_ = np.arange(128)
        k_ = np.arange(K17)
        t_ = np.arange(NT)
        idx = (s0 - WIN + 128 * t_[None, None, :] + p_[:, None, None]
               + k_[None, :, None])                       # [128, K17, NT]
        okp = (idx >= 0) & (idx < N)
        po_g = np.where(okp[..., None], po_full[np.clip(idx, 0, N - 1)], 0.0)
        po_sh = po_g.reshape(128, K17 * NT * 12).astype(np.float32)

        xc = xf[s0:s0 + NPC].reshape(NT, 128, C).transpose(1, 0, 2) \
            .reshape(128, NT * C).astype(np.float32)

        # chain-boundary masks
        n_ = s0 + 128 * t_[None, :] + p_[:, None]          # [128, NT]
        off = n_ % L
        kk = np.arange(-WIN, WIN + 1)
        v = ((off[..., None] + kk) >= 0) & ((off[..., None] + kk) < L)  # [128,NT,K17]
        mask = np.broadcast_to(v[..., None].astype(np.float32),
                               (128, NT, K17, 8)).reshape(128, NT * K17 * 8)
        ncl = (K17 - v.sum(-1)).astype(np.float32)

        in_maps.append(dict(
            xT_h=xT_h, po_sh=po_sh, xc=xc,
            maskd=mask.astype(bfloat16), nclmp=ncl, **common))
    return in_maps


def kernel(x, pos, seq, ori, W_in, Ws0, bs0, Wk, W_out, src, dst):
    exp_src, exp_dst = _expected_src_dst()
    assert np.array_equal(np.asarray(src), exp_src), "unexpected src graph"
    assert np.array_equal(np.asarray(dst), exp_dst), "unexpected dst graph"

    from concourse.bass_utils import run_bass_kernel_spmd

    if "nc" not in _PROG:
        _PROG["nc"] = _build_program()
    nc = _PROG["nc"]

    in_maps = _host_inputs(np.asarray(x), np.asarray(pos), np.asarray(ori),
                           np.asarray(W_in), np.asarray(Ws0), np.asarray(bs0),
                           np.asarray(Wk), np.asarray(W_out))
    res = run_bass_kernel_spmd(nc, in_maps, list(range(NCORES)))
    out = np.concatenate([res.results[i]["y"] for i in range(NCORES)], axis=0)
    return out.reshape(B, L, C).astype(np.float32)
